# revision 40
# baseline (speedup 1.0000x reference)
"""AttentionWeightedAverage distributed Trainium2 kernel.

Reference computation (all f32):
    s     = wv @ v + wg @ h          # (512, 384) + (512, 1) broadcast
    t     = tanh(s)                  # (512, 384)
    z     = wh @ t                   # (384, 384)
    alpha = softmax(z, axis=-1)      # (384, 384)
    out[i, j, l] = v[j, l] * alpha[i, j]   # (384, 384, 384)

The output dominates (226 MB f32 vs ~2.5 MB of inputs): the kernel is
bound by the HBM store stream. The rel-err gate is 2e-2, so the device
writes the output in BF16 (adds <= ~0.4% rel-to-max; measured total
5.7e-3) and the host upcasts to f32 — this halves the store stream to
14.45 MB per core ≈ 40 us at the ~358 GB/s per-core HBM limit, and is
the single biggest win over an f32 store (80 us). fp8 was evaluated and
rejected: the softmax is nearly flat (max alpha ~2%), so >0.2-max "hot"
rows land in 100% of 128-row j-tiles and no static mixed-precision
routing is safe.

Sharding: every core gets the full (small) weights and computes s/t
redundantly; core m owns rows i in [m*48, (m+1)*48) of z/alpha and
writes that contiguous slice of the output. No collectives.

The prologue (everything before alpha) is latency-critical (~13 us of
PE chain after a ~7 us fixed framework preamble):
- matmul operands bf16 (fast LDWEIGHTS); PSUM accumulation f32.
- inputs ride 2 fused DMAs per HWDGE queue ([wvT|vb] chunks on SP,
  [h|wgT] + [whT|v3] on ACT) — the ~0.6 us per-DMA issue cost on the
  engines was serializing the load phase when split into 8 DMAs.
- 8 throwaway matmuls on zeroed tiles ramp the PE clock (HAM needs
  ~5 us of sustained activity; unramped matmuls run ~1.6x slow) while
  the loads are in flight.
- z is one 48-row pass: the N=384 matmul cost is row-count-independent,
  so the old 2x24-row split doubled the z PE time for nothing.
- softmax skips the max-subtraction (|z| small, shift-invariant), row
  sums come free via exp's accum_out. The normalize/transpose/copy
  chain runs as two passes (3 DVE norms back-to-back, then PE
  transposes pipelined against DVE copies) -- the fused per-c loop
  ping-ponged DVE<->PE serially (~1.95us vs ~1.3us to first multiply).
- alpha is normalized into BF16: the PE transpose of a bf16 input is
  one-pass (~195-220ns vs 340ns for f32 LOW_HIGH two-pass) and the
  bf16-out normalize hits DVE 4x. Costs ~+0.26% rel err (8.3e-3 total
  on the fixed seed, 2.4x under the gate; fresh-seed check 9.2e-3).
- the broadcast multiply splits (i,c) tiles 2:1 DVE:ACT — bf16 DVE
  tensor_scalar runs the 4x uop mode (~266 ns/tile vs ACT ~500 ns).

Measured on trn2 (8 cores, axon NTFF profile), exec_time_ns: min ~61.4,
good-cluster 62-64 (5/6 runs in the final batch), tail to ~72 us under
the straggler/load-jitter lotteries. Breakdown: ~6 us counted preamble,
first multiply at ~18.5-22 us, 37-40 us store stream saturated at
385-389 GB/s with zero mid-stream dips, then a fixed ~8.8 us compiler
(walrus) postamble: each engine individually clears its ~50-semaphore
block of the full 256-sem file; the Tensor sequencer does ~115 ns per
clear (~5.6 us serial) while every other engine waits at the final
barrier. Not controllable from kernel code.

Variance sources (both environmental, not routable from the kernel):
- SDMA engine 15 (E79) intermittently runs ~10% slow and finishes
  0-7 us behind the other 15 engines (known engine-7/15 issue); store
  bytes are split statically 1/16 per engine per InstDMACopy.
- The 8-core input inrush (8 x 1.84 MB of replicated weights) is
  chip-HBM-bound (~5.1 us floor); per-core load completion varies
  10.5-15 us on arbitration luck, and a late chunk stalls the in-order
  PE chain + resets the HAM ramp (~+2.5 us cascade).
Do NOT split the first output row into per-chunk DMAs: 768 B-descriptor
DMAs at the queue head stall the stream ramp ~4 us (measured +6 us
median). Do NOT raise IPB: store-block granularity must stay under
production-rate x drain-time or the queue runs dry early (IPB=4 needs
12 tiles ~3.7 us per block vs 3.1 us drain -> 180-230 GB/s dips,
measured +7 us median). IPB=2 (6 tiles ~1.9 us vs 1.5 us drain) is the
sweet spot; IPB=1 would push the SP engine to ~84% issue duty.

Per-core SBUF layouts (P = 128 partitions):
    wvvb (128, 3*896) bf16: per k chunk [wvT | vb]:
          wvT[p, k*896+e]     = wv[e, k*128+p]
          vb [p, k*896+512+l] = v[k*128+p, l]      (s-matmul operands)
    hwg  (128, 4+2048) bf16: [h3 | wgT3]; h3[p,k]=h[k*128+p],
          wgT3[p, k*512+e] = wg[e, k*128+p]
    wv3  (128, 192+1152) bf16: [whT3 | v3];
          whT3[p, k*48+i] = wh[m*48+i, k*128+p]
          v3[p, c*384+l]  = v[3p+c, l]  (broadcast source, layout B:
          partition p's 3 j-rows are consecutive in DRAM -> 2.3 KB
          contiguous runs per store descriptor row)
"""

import os

import numpy as np

import concourse.bacc as bacc
import concourse.mybir as mybir
from concourse import masks
from concourse.bass_utils import run_bass_kernel_spmd
from concourse.tile import TileContext

F32 = mybir.dt.float32
BF16 = mybir.dt.bfloat16
FP16 = mybir.dt.float16
AF = mybir.ActivationFunctionType

NCORES = 8
L = 384          # vfeat_len == vfeat_dim
E = 512          # embed dim
IPC = L // NCORES  # 48 output rows per core
P = 128
CJ = L // P      # 3 chunks over the j axis
KV = L // P      # 3 contraction chunks for wv@v
KE = E // P      # 4 contraction chunks over embed dim
IPB = int(os.environ.get("K_IPB", "2"))  # output rows batched per store DMA
OUT_BUFS = int(os.environ.get("K_OUT_BUFS", "8"))  # in-flight output tiles

# Output is stored 16-bit (rel-err gate is 2e-2; bf16 store adds ~0.4%
# rel-to-max) and upcast to f32 on the host: halves the HBM store
# stream, which is the roofline for this kernel.
_DT = {"bf16": BF16, "fp16": FP16, "f32": F32}
OUT_DT = _DT[os.environ.get("K_OUT_DT", "bf16")]
# multiply tiles with (i*CJ+c) % ACT_MOD >= ACT_CUT go to ACT, rest DVE
# (bf16 DVE runs 4x mode ~266ns/tile; ACT ~500ns/tile -> 2:1 split)
ACT_MOD = int(os.environ.get("K_ACT_MOD", "3"))
ACT_CUT = int(os.environ.get("K_ACT_CUT", "2"))
NWARM = int(os.environ.get("K_NWARM", "8"))
EL = E + L        # fused wvT|vb chunk width


def _build_nc() -> bacc.Bacc:
    nc = bacc.Bacc()

    # wvvb: per k-chunk [wvT chunk (512) | vb chunk (384)] interleaved so
    # the s-matmul operands arrive in 2 large DMAs instead of 6 (the SP
    # engine's ~0.6us per-DMA issue cost was serializing the load phase).
    wvvb_d = nc.declare_dram_parameter("wvvb", [P, KV * EL], BF16, isOutput=False)
    hwg_d = nc.declare_dram_parameter("hwg", [P, KE + KE * E], BF16, isOutput=False)
    # wv3: [whT3 | v3 (broadcast source, layout B)] single ACT-queue DMA
    wv3_d = nc.declare_dram_parameter(
        "wv3", [P, KE * IPC + CJ * L], BF16, isOutput=False
    )
    out_d = nc.declare_dram_parameter("out", [IPC, L, L], OUT_DT, isOutput=True)

    with TileContext(nc) as tc:
        with (
            tc.tile_pool(name="const", bufs=1) as cpool,
            tc.tile_pool(name="work", bufs=2) as wpool,
            tc.tile_pool(name="psum", bufs=2, space="PSUM") as ppool,
            tc.tile_pool(name="outp", bufs=OUT_BUFS) as opool,
        ):
            # ---- input loads; split across the two HWDGE queues (SP + ACT)
            # and chunked along K so dependent matmuls start per-chunk.
            # part 1 covers h + wgT k0..k1 so the first two gh matmuls are
            # gated on an early 268KB transfer; part 2 (k2..k3) has until
            # after s-mc0/mc1 to land (absorbs HBM load-phase contention
            # jitter -- a PE idle gap here resets the HAM clock ramp).
            hwg_sb = cpool.tile([P, KE + KE * E], BF16)
            nc.scalar.dma_start(
                out=hwg_sb[:, 0 : KE + 2 * E], in_=hwg_d[:, 0 : KE + 2 * E]
            )
            nc.scalar.dma_start(
                out=hwg_sb[:, KE + 2 * E :], in_=hwg_d[:, KE + 2 * E :]
            )
            h_sb = hwg_sb[:, 0:KE]
            wgT_sb = hwg_sb[:, KE:]
            wvvb_sb = cpool.tile([P, KV * EL], BF16)
            nc.sync.dma_start(out=wvvb_sb[:, 0:EL], in_=wvvb_d[:, 0:EL])
            nc.sync.dma_start(out=wvvb_sb[:, EL:], in_=wvvb_d[:, EL:])
            # whT + v3 ride the ACT queue behind hwg so the SP queue only
            # carries the s-matmul operands.
            wv3_sb = cpool.tile([P, KE * IPC + CJ * L], BF16)
            nc.scalar.dma_start(out=wv3_sb[:], in_=wv3_d[:])
            whT_sb = wv3_sb[:, 0 : KE * IPC]
            v_sb = wv3_sb[:, KE * IPC :]

            ident = cpool.tile([IPC, IPC], F32)
            masks.make_identity(nc, ident[:])
            # bf16 identity for the alpha transposes: bf16 input makes
            # the PE transpose one-pass (f32 runs LOW_HIGH = 2 passes,
            # 340ns vs ~210ns) and the bf16-out normalize hits DVE 4x.
            # alpha in bf16 adds ~0.2% rel err (margin stays ~2.5x).
            identb = cpool.tile([IPC, IPC], BF16)
            masks.make_identity(nc, identb[:])

            # Warm the PE (HAM throttle needs ~4us of sustained matmul
            # activity to reach full clock) with throwaway matmuls on
            # zeroed tiles while the weight DMAs are still in flight.
            warm_w = cpool.tile([P, P], BF16)
            warm_x = cpool.tile([P, L], BF16)
            nc.gpsimd.memset(warm_w[:], 0.0)
            nc.gpsimd.memset(warm_x[:], 0.0)
            warm_ps = ppool.tile([P, L], F32, tag="s_ps", bufs=KE)
            for w in range(NWARM):
                nc.tensor.matmul(
                    warm_ps[:],
                    lhsT=warm_w[:],
                    rhs=warm_x[:],
                    start=(w == 0),
                    stop=(w == NWARM - 1),
                )

            # ---- ghT[0, e] = (wg @ h)[e], e in 0..511.  The k2/k3 chunks
            # are emitted AFTER s-mc0/mc1 so the PE never idles waiting on
            # hwg part 2 (a PE gap resets the HAM clock ramp and the whole
            # downstream chain then runs ~1.6x slow).
            ghT_ps = ppool.tile([1, E], F32, tag="zg", bufs=2)
            for k in (0, 1):
                nc.tensor.matmul(
                    ghT_ps[:],
                    lhsT=h_sb[:, k : k + 1],
                    rhs=wgT_sb[:, k * E : (k + 1) * E],
                    start=(k == 0),
                    stop=False,
                )

            # ---- s = wv @ v per mc chunk (tanh applied once gh is ready)
            t3 = cpool.tile([P, KE * L], BF16)

            def s_matmuls(mc):
                s_ps = ppool.tile([P, L], F32, tag="s_ps", bufs=KE)
                for k in range(KV):
                    nc.tensor.matmul(
                        s_ps[:],
                        lhsT=wvvb_sb[:, k * EL + mc * P : k * EL + (mc + 1) * P],
                        rhs=wvvb_sb[:, k * EL + E : (k + 1) * EL],
                        start=(k == 0),
                        stop=(k == KV - 1),
                    )
                return s_ps

            s_early = [s_matmuls(0), s_matmuls(1)]

            for k in (2, 3):
                nc.tensor.matmul(
                    ghT_ps[:],
                    lhsT=h_sb[:, k : k + 1],
                    rhs=wgT_sb[:, k * E : (k + 1) * E],
                    start=False,
                    stop=(k == KE - 1),
                )
            ghT_sb = wpool.tile([1, E], F32)
            nc.vector.tensor_copy(ghT_sb[:], ghT_ps[:])
            # reorient via 4 tiny K=1 PE transposes: gh_sb[p,mc]=gh[mc*128+p]
            gh_sb = wpool.tile([P, KE], F32)
            for mc in range(KE):
                gt_ps = ppool.tile([P, 1], F32, tag="at_ps")
                nc.tensor.transpose(
                    gt_ps[:], ghT_sb[:, mc * P : (mc + 1) * P], ident[0:1, 0:1]
                )
                nc.vector.tensor_copy(gh_sb[:, mc : mc + 1], gt_ps[:])

            # ---- t = tanh(s + gh . 1^T), t3[p, mc*384+j] = t[mc*128+p, j]
            for mc in range(KE):
                s_ps = s_early[mc] if mc < 2 else s_matmuls(mc)
                nc.scalar.activation(
                    t3[:, mc * L : (mc + 1) * L], s_ps[:], AF.Tanh,
                    bias=gh_sb[:, mc : mc + 1], scale=1.0,
                )

            # ---- z (all 48 rows in one pass: the N=384 matmul cost is
            # independent of the row count, so splitting into halves only
            # doubled the PE work), softmax, transpose.
            alphaT = wpool.tile([P, CJ * IPC], F32)
            z_h = ppool.tile([IPC, L], F32, tag="zg", bufs=2)
            for k in range(KE):
                nc.tensor.matmul(
                    z_h[:],
                    lhsT=whT_sb[:, k * IPC : (k + 1) * IPC],
                    rhs=t3[:, k * L : (k + 1) * L],
                    start=(k == 0),
                    stop=(k == KE - 1),
                )
            # softmax (no max shift; fused row sums)
            e_h = wpool.tile([IPC, L], F32, tag="e_h")
            rsum_h = wpool.tile([IPC, 1], F32, tag="rsum_h")
            nc.scalar.activation(e_h[:], z_h[:], AF.Exp, accum_out=rsum_h[:])
            rinv_h = wpool.tile([IPC, 1], F32, tag="rinv_h")
            nc.vector.reciprocal(rinv_h[:], rsum_h[:])
            # alphaT[p, c*48+i] = alpha[i, 3p+c]; the DVE normalize
            # also performs the stride-3 column gather (j = 3p+c) so
            # the PE transpose reads a contiguous slice. (Tried on ACT
            # to shorten the DVE queue: ACT's strided gather is slower
            # and serializes behind exp -- first_mul regressed ~1.5us.)
            # two passes: all 3 norms run back-to-back on DVE, then the
            # PE transposes pipeline against the DVE copies -- the fused
            # per-c loop ping-ponged DVE<->PE serially (~1.95us vs ~1.3).
            alpha_h = wpool.tile([IPC, L], BF16, tag="alpha_h")
            for c in range(CJ):
                nc.vector.tensor_scalar_mul(
                    alpha_h[:, c * P : (c + 1) * P],
                    e_h.rearrange("i (p c) -> c i p", c=CJ)[c],
                    rinv_h[:],
                )
            for c in range(CJ):
                at_ps = ppool.tile([P, IPC], BF16, tag="at_ps")
                nc.tensor.transpose(
                    at_ps[:],
                    alpha_h[:, c * P : (c + 1) * P],
                    identb[0:IPC, 0:IPC],
                )
                nc.vector.tensor_copy(
                    alphaT[:, c * IPC : (c + 1) * IPC], at_ps[:]
                )

            # ---- out[i, c*128+p, l] = v[c*128+p, l] * alpha[i, c*128+p]
            # First block is a single row so the store stream starts as
            # early as possible; the rest are IPB-row blocks.
            blocks = [(0, 1), (1, 1), (2, 1), (3, 1)] + [
                (ib, min(IPB, IPC - ib)) for ib in range(4, IPC, IPB)
            ]
            for ib, nb in blocks:
                ot = opool.tile([P, IPB * CJ * L], OUT_DT, tag="ot")
                for t in range(nb):
                    i = ib + t
                    for c in range(CJ):
                        dst = ot[:, (t * CJ + c) * L : (t * CJ + c + 1) * L]
                        src = v_sb[:, c * L : (c + 1) * L]
                        sc = alphaT[:, c * IPC + i : c * IPC + i + 1]
                        if i == 0 and c == 1:
                            # first row: split DVE/ACT so the first store
                            # DMA fires as early as possible (gpsimd is
                            # ~14x slower here and stalls DVE via the
                            # shared SBUF ports - do not use it). Do NOT
                            # split this row into per-chunk DMAs: the
                            # 768B-descriptor DMAs stall the queue head
                            # for ~4us (descriptor-dominated, FIFO ahead
                            # of the big blocks) - measured +6us median.
                            nc.scalar.mul(dst, src, sc)
                        elif (i * CJ + c) % ACT_MOD < ACT_CUT:
                            nc.vector.tensor_scalar_mul(dst, src, sc)
                        else:
                            nc.scalar.mul(dst, src, sc)
                dram_ap = out_d[ib : ib + nb].rearrange(
                    "t (p c) l -> p t c l", p=P, c=CJ
                )
                sb_ap = ot[:, 0 : nb * CJ * L].rearrange(
                    "p (t c l) -> p t c l", t=nb, c=CJ
                )
                # (tried issuing block 0 from the ACT ring to skip the
                # cross-engine sem hop: ACT's issue actually lands ~1us
                # LATER and the ACT ring's first-byte DGE latency is
                # ~1.3us vs SP's ~0.6us -- no gain, reverted)
                nc.sync.dma_start(out=dram_ap, in_=sb_ap)

    nc.compile()
    return nc


def _prep_inputs(h, v, wh, wv, wg):
    """Host-side relayout into the per-core SBUF-friendly layouts."""
    h = np.ascontiguousarray(h, dtype=np.float32)
    v = np.ascontiguousarray(v, dtype=np.float32)
    wh = np.ascontiguousarray(wh, dtype=np.float32)
    wv = np.ascontiguousarray(wv, dtype=np.float32)
    wg = np.ascontiguousarray(wg, dtype=np.float32)

    def bf16(x):
        import ml_dtypes

        return np.ascontiguousarray(x.astype(ml_dtypes.bfloat16))

    # v3 (broadcast source): layout B, v3[p, c*384+l] = v[3p+c, l]
    # so each partition's 3 rows are CONSECUTIVE in the output -> 2.3 KB
    # contiguous HBM runs per store-DMA descriptor row.
    v3 = v.reshape(P, CJ * L)
    # vA (matmul rhs): layout A, vA[p, k*384+l] = v[k*128+p, l]
    vA = v.reshape(CJ, P, L).transpose(1, 0, 2).reshape(P, CJ, L)
    wvT3 = wv.T.reshape(KV, P, E).transpose(1, 0, 2)  # (P, KV, E)
    # fused [wvT chunk | vb chunk] per k
    wvvb = bf16(np.concatenate([wvT3, vA], axis=2).reshape(P, KV * EL))
    wgT3 = wg.T.reshape(KE, P, E).transpose(1, 0, 2).reshape(P, KE * E)
    hwg = bf16(np.concatenate([h.reshape(KE, P).T, wgT3], axis=1))

    in_maps = []
    for m in range(NCORES):
        whm = wh[m * IPC : (m + 1) * IPC]  # (48, 512)
        whT3 = whm.T.reshape(KE, P, IPC).transpose(1, 0, 2).reshape(P, KE * IPC)
        wv3 = bf16(np.concatenate([whT3, v3], axis=1))
        in_maps.append(
            {
                "wvvb": wvvb,
                "hwg": hwg,
                "wv3": wv3,
            }
        )
    return in_maps


_NC_CACHE = []


def _run(inputs: dict, trace: bool = False, **kw):
    if not _NC_CACHE:
        _NC_CACHE.append(_build_nc())
    nc = _NC_CACHE[0]
    in_maps = _prep_inputs(**inputs)
    res = run_bass_kernel_spmd(
        nc, in_maps, core_ids=list(range(NCORES)), trace=trace, **kw
    )
    # 16-bit device output -> f32 on the host (lossless upcast; the
    # device computed every output value, this is just representation).
    out = np.concatenate(
        [np.asarray(r["out"]).astype(np.float32) for r in res.results], axis=0
    )
    return out, res


def kernel(h, v, wh, wv, wg):
    out, _ = _run({"h": h, "v": v, "wh": wh, "wv": wv, "wg": wg})
    return out



# revision 43
# speedup vs baseline: 1.0496x; 1.0496x over previous
"""AttentionWeightedAverage distributed Trainium2 kernel.

Reference computation (all f32):
    s     = wv @ v + wg @ h          # (512, 384) + (512, 1) broadcast
    t     = tanh(s)                  # (512, 384)
    z     = wh @ t                   # (384, 384)
    alpha = softmax(z, axis=-1)      # (384, 384)
    out[i, j, l] = v[j, l] * alpha[i, j]   # (384, 384, 384)

The output dominates (226 MB f32 vs ~2.5 MB of inputs): the kernel is
bound by the HBM store stream. The rel-err gate is 2e-2, so the device
writes the output in BF16 (adds <= ~0.4% rel-to-max; measured total
5.7e-3) and the host upcasts to f32 — this halves the store stream to
14.45 MB per core ≈ 40 us at the ~358 GB/s per-core HBM limit, and is
the single biggest win over an f32 store (80 us). fp8 was evaluated and
rejected: the softmax is nearly flat (max alpha ~2%), so >0.2-max "hot"
rows land in 100% of 128-row j-tiles and no static mixed-precision
routing is safe.

Sharding: every core gets the full (small) weights and computes s/t
redundantly; core m owns rows i in [m*48, (m+1)*48) of z/alpha and
writes that contiguous slice of the output. No collectives.

The prologue (everything before alpha) is latency-critical (~13 us of
PE chain after a ~7 us fixed framework preamble):
- matmul operands bf16 (fast LDWEIGHTS); PSUM accumulation f32.
- inputs ride 2 fused DMAs per HWDGE queue ([wvT|vb] chunks on SP,
  [h|wgT] + [whT|v3] on ACT) — the ~0.6 us per-DMA issue cost on the
  engines was serializing the load phase when split into 8 DMAs.
- 8 throwaway matmuls on zeroed tiles ramp the PE clock (HAM needs
  ~5 us of sustained activity; unramped matmuls run ~1.6x slow) while
  the loads are in flight.
- z is one 48-row pass: the N=384 matmul cost is row-count-independent,
  so the old 2x24-row split doubled the z PE time for nothing.
- softmax skips the max-subtraction (|z| small, shift-invariant), row
  sums come free via exp's accum_out. The normalize/transpose/copy
  chain runs as two passes (3 DVE norms back-to-back, then PE
  transposes pipelined against DVE copies) -- the fused per-c loop
  ping-ponged DVE<->PE serially (~1.95us vs ~1.3us to first multiply).
- alpha is normalized into BF16: the PE transpose of a bf16 input is
  one-pass (~195-220ns vs 340ns for f32 LOW_HIGH two-pass) and the
  bf16-out normalize hits DVE 4x. Costs ~+0.26% rel err (8.3e-3 total
  on the fixed seed, 2.4x under the gate; fresh-seed check 9.2e-3).
- the broadcast multiply splits (i,c) tiles 2:1 DVE:ACT — bf16 DVE
  tensor_scalar runs the 4x uop mode (~266 ns/tile vs ACT ~500 ns).

Measured on trn2 (8 cores, axon NTFF profile), exec_time_ns: min ~61.4,
good-cluster 62-64 (5/6 runs in the final batch), tail to ~72 us under
the straggler/load-jitter lotteries. Breakdown: ~6 us counted preamble,
first multiply at ~18.5-22 us, 37-40 us store stream saturated at
385-389 GB/s with zero mid-stream dips, then a fixed ~8.8 us compiler
(walrus) postamble: each engine individually clears its ~50-semaphore
block of the full 256-sem file; the Tensor sequencer does ~115 ns per
clear (~5.6 us serial) while every other engine waits at the final
barrier. Not controllable from kernel code.

Variance sources (both environmental, not routable from the kernel):
- SDMA engine 15 (E79) intermittently runs ~10% slow and finishes
  0-7 us behind the other 15 engines (known engine-7/15 issue); store
  bytes are split statically 1/16 per engine per InstDMACopy.
- The 8-core input inrush (8 x 1.84 MB of replicated weights) is
  chip-HBM-bound (~5.1 us floor); per-core load completion varies
  10.5-15 us on arbitration luck, and a late chunk stalls the in-order
  PE chain + resets the HAM ramp (~+2.5 us cascade).
Do NOT split the first output row into per-chunk DMAs: 768 B-descriptor
DMAs at the queue head stall the stream ramp ~4 us (measured +6 us
median). Do NOT raise IPB: store-block granularity must stay under
production-rate x drain-time or the queue runs dry early (IPB=4 needs
12 tiles ~3.7 us per block vs 3.1 us drain -> 180-230 GB/s dips,
measured +7 us median). IPB=2 (6 tiles ~1.9 us vs 1.5 us drain) is the
sweet spot; IPB=1 would push the SP engine to ~84% issue duty.

Per-core SBUF layouts (P = 128 partitions):
    wvvb (128, 3*896) bf16: per k chunk [wvT | vb]:
          wvT[p, k*896+e]     = wv[e, k*128+p]
          vb [p, k*896+512+l] = v[k*128+p, l]      (s-matmul operands)
    hwg  (128, 4+2048) bf16: [h3 | wgT3]; h3[p,k]=h[k*128+p],
          wgT3[p, k*512+e] = wg[e, k*128+p]
    wv3  (128, 192+1152) bf16: [whT3 | v3];
          whT3[p, k*48+i] = wh[m*48+i, k*128+p]
          v3[p, c*384+l]  = v[3p+c, l]  (broadcast source, layout B:
          partition p's 3 j-rows are consecutive in DRAM -> 2.3 KB
          contiguous runs per store descriptor row)
"""

import os

import numpy as np

import concourse.bacc as bacc
import concourse.mybir as mybir
from concourse import masks
from concourse.bass_utils import run_bass_kernel_spmd
from concourse.tile import TileContext

F32 = mybir.dt.float32
BF16 = mybir.dt.bfloat16
FP16 = mybir.dt.float16
AF = mybir.ActivationFunctionType

NCORES = 8
L = 384          # vfeat_len == vfeat_dim
E = 512          # embed dim
IPC = L // NCORES  # 48 output rows per core
P = 128
CJ = L // P      # 3 chunks over the j axis
KV = L // P      # 3 contraction chunks for wv@v
KE = E // P      # 4 contraction chunks over embed dim
IPB = int(os.environ.get("K_IPB", "2"))  # output rows batched per store DMA
OUT_BUFS = int(os.environ.get("K_OUT_BUFS", "8"))  # in-flight output tiles

# Output is stored 16-bit (rel-err gate is 2e-2; bf16 store adds ~0.4%
# rel-to-max) and upcast to f32 on the host: halves the HBM store
# stream, which is the roofline for this kernel.
_DT = {"bf16": BF16, "fp16": FP16, "f32": F32}
OUT_DT = _DT[os.environ.get("K_OUT_DT", "bf16")]
# multiply tiles with (i*CJ+c) % ACT_MOD >= ACT_CUT go to ACT, rest DVE
# (bf16 DVE runs 4x mode ~266ns/tile; ACT ~500ns/tile -> 2:1 split)
ACT_MOD = int(os.environ.get("K_ACT_MOD", "3"))
ACT_CUT = int(os.environ.get("K_ACT_CUT", "2"))
NWARM = int(os.environ.get("K_NWARM", "8"))
EL = E + L        # fused wvT|vb chunk width


def _build_nc() -> bacc.Bacc:
    nc = bacc.Bacc()

    # wvvb: per k-chunk [wvT chunk (512) | vb chunk (384)] interleaved so
    # the s-matmul operands arrive in 2 large DMAs instead of 6 (the SP
    # engine's ~0.6us per-DMA issue cost was serializing the load phase).
    wvvb_d = nc.declare_dram_parameter("wvvb", [P, KV * EL], BF16, isOutput=False)
    hwg_d = nc.declare_dram_parameter("hwg", [P, KE + KE * E], BF16, isOutput=False)
    # wv3: [whT3 | v3 (broadcast source, layout B)] single ACT-queue DMA
    wv3_d = nc.declare_dram_parameter(
        "wv3", [P, KE * IPC + CJ * L], BF16, isOutput=False
    )
    out_d = nc.declare_dram_parameter("out", [IPC, L, L], OUT_DT, isOutput=True)

    with TileContext(nc) as tc:
        with (
            tc.tile_pool(name="const", bufs=1) as cpool,
            tc.tile_pool(name="work", bufs=2) as wpool,
            tc.tile_pool(name="psum", bufs=2, space="PSUM") as ppool,
            tc.tile_pool(name="outp", bufs=OUT_BUFS) as opool,
        ):
            # ---- input loads; split across the two HWDGE queues (SP + ACT)
            # and chunked along K so dependent matmuls start per-chunk.
            # part 1 covers h + wgT k0..k1 so the first two gh matmuls are
            # gated on an early 268KB transfer; part 2 (k2..k3) has until
            # after s-mc0/mc1 to land (absorbs HBM load-phase contention
            # jitter -- a PE idle gap here resets the HAM clock ramp).
            hwg_sb = cpool.tile([P, KE + KE * E], BF16)
            nc.scalar.dma_start(
                out=hwg_sb[:, 0 : KE + 2 * E], in_=hwg_d[:, 0 : KE + 2 * E]
            )
            nc.scalar.dma_start(
                out=hwg_sb[:, KE + 2 * E :], in_=hwg_d[:, KE + 2 * E :]
            )
            h_sb = hwg_sb[:, 0:KE]
            wgT_sb = hwg_sb[:, KE:]
            wvvb_sb = cpool.tile([P, KV * EL], BF16)
            nc.sync.dma_start(out=wvvb_sb[:, 0:EL], in_=wvvb_d[:, 0:EL])
            nc.sync.dma_start(out=wvvb_sb[:, EL:], in_=wvvb_d[:, EL:])
            # whT + v3 ride the ACT queue behind hwg so the SP queue only
            # carries the s-matmul operands.
            wv3_sb = cpool.tile([P, KE * IPC + CJ * L], BF16)
            nc.scalar.dma_start(out=wv3_sb[:], in_=wv3_d[:])
            whT_sb = wv3_sb[:, 0 : KE * IPC]
            v_sb = wv3_sb[:, KE * IPC :]

            ident = cpool.tile([IPC, IPC], F32)
            masks.make_identity(nc, ident[:])
            # bf16 identity for the alpha transposes: bf16 input makes
            # the PE transpose one-pass (f32 runs LOW_HIGH = 2 passes,
            # 340ns vs ~210ns) and the bf16-out normalize hits DVE 4x.
            # alpha in bf16 adds ~0.2% rel err (margin stays ~2.5x).
            identb = cpool.tile([IPC, IPC], BF16)
            masks.make_identity(nc, identb[:])

            # Warm the PE (HAM throttle needs ~4us of sustained matmul
            # activity to reach full clock) with throwaway matmuls on
            # zeroed tiles while the weight DMAs are still in flight.
            warm_w = cpool.tile([P, P], BF16)
            warm_x = cpool.tile([P, L], BF16)
            nc.gpsimd.memset(warm_w[:], 0.0)
            nc.gpsimd.memset(warm_x[:], 0.0)
            warm_ps = ppool.tile([P, L], F32, tag="s_ps", bufs=KE)
            for w in range(NWARM):
                nc.tensor.matmul(
                    warm_ps[:],
                    lhsT=warm_w[:],
                    rhs=warm_x[:],
                    start=(w == 0),
                    stop=(w == NWARM - 1),
                )

            # ---- ghT[0, e] = (wg @ h)[e], e in 0..511.  The k2/k3 chunks
            # are emitted AFTER s-mc0/mc1 so the PE never idles waiting on
            # hwg part 2 (a PE gap resets the HAM clock ramp and the whole
            # downstream chain then runs ~1.6x slow).
            ghT_ps = ppool.tile([1, E], F32, tag="zg", bufs=2)
            for k in (0, 1):
                nc.tensor.matmul(
                    ghT_ps[:],
                    lhsT=h_sb[:, k : k + 1],
                    rhs=wgT_sb[:, k * E : (k + 1) * E],
                    start=(k == 0),
                    stop=False,
                )

            # ---- s = wv @ v per mc chunk (tanh applied once gh is ready)
            t3 = cpool.tile([P, KE * L], BF16)

            def s_matmuls(mc):
                s_ps = ppool.tile([P, L], F32, tag="s_ps", bufs=KE)
                for k in range(KV):
                    nc.tensor.matmul(
                        s_ps[:],
                        lhsT=wvvb_sb[:, k * EL + mc * P : k * EL + (mc + 1) * P],
                        rhs=wvvb_sb[:, k * EL + E : (k + 1) * EL],
                        start=(k == 0),
                        stop=(k == KV - 1),
                    )
                return s_ps

            s_early = [s_matmuls(0), s_matmuls(1)]

            for k in (2, 3):
                nc.tensor.matmul(
                    ghT_ps[:],
                    lhsT=h_sb[:, k : k + 1],
                    rhs=wgT_sb[:, k * E : (k + 1) * E],
                    start=False,
                    stop=(k == KE - 1),
                )
            ghT_sb = wpool.tile([1, E], F32)
            nc.vector.tensor_copy(ghT_sb[:], ghT_ps[:])
            # reorient via 4 tiny K=1 PE transposes: gh_sb[p,mc]=gh[mc*128+p]
            gh_sb = wpool.tile([P, KE], F32)
            for mc in range(KE):
                gt_ps = ppool.tile([P, 1], F32, tag="at_ps")
                nc.tensor.transpose(
                    gt_ps[:], ghT_sb[:, mc * P : (mc + 1) * P], ident[0:1, 0:1]
                )
                nc.vector.tensor_copy(gh_sb[:, mc : mc + 1], gt_ps[:])

            # ---- t = tanh(s + gh . 1^T), t3[p, mc*384+j] = t[mc*128+p, j]
            for mc in range(KE):
                s_ps = s_early[mc] if mc < 2 else s_matmuls(mc)
                nc.scalar.activation(
                    t3[:, mc * L : (mc + 1) * L], s_ps[:], AF.Tanh,
                    bias=gh_sb[:, mc : mc + 1], scale=1.0,
                )

            # ---- z (all 48 rows in one pass: the N=384 matmul cost is
            # independent of the row count, so splitting into halves only
            # doubled the PE work), softmax, transpose.
            alphaT = wpool.tile([P, CJ * IPC], F32)
            z_h = ppool.tile([IPC, L], F32, tag="zg", bufs=2)
            for k in range(KE):
                nc.tensor.matmul(
                    z_h[:],
                    lhsT=whT_sb[:, k * IPC : (k + 1) * IPC],
                    rhs=t3[:, k * L : (k + 1) * L],
                    start=(k == 0),
                    stop=(k == KE - 1),
                )
            # softmax (no max shift; fused row sums)
            e_h = wpool.tile([IPC, L], F32, tag="e_h")
            rsum_h = wpool.tile([IPC, 1], F32, tag="rsum_h")
            nc.scalar.activation(e_h[:], z_h[:], AF.Exp, accum_out=rsum_h[:])
            rinv_h = wpool.tile([IPC, 1], F32, tag="rinv_h")
            nc.vector.reciprocal(rinv_h[:], rsum_h[:])
            # alphaT[p, c*48+i] = alpha[i, 3p+c]; the DVE normalize
            # also performs the stride-3 column gather (j = 3p+c) so
            # the PE transpose reads a contiguous slice. (Tried on ACT
            # to shorten the DVE queue: ACT's strided gather is slower
            # and serializes behind exp -- first_mul regressed ~1.5us.)
            # two passes: all 3 norms run back-to-back on DVE, then the
            # PE transposes pipeline against the DVE copies -- the fused
            # per-c loop ping-ponged DVE<->PE serially (~1.95us vs ~1.3).
            # per-c normalizes (NOT one fused 384-col op: fused runs
            # 709ns and delays transpose-c0 by ~0.4us -- the block-0
            # critical path wants c0/c1 transposed ASAP; latency beats
            # the ~160ns of DVE busy a fused op would save)
            alpha_h = wpool.tile([IPC, L], BF16, tag="alpha_h")
            for c in range(CJ):
                nc.vector.tensor_scalar_mul(
                    alpha_h[:, c * P : (c + 1) * P],
                    e_h.rearrange("i (p c) -> c i p", c=CJ)[c],
                    rinv_h[:],
                )
            for c in range(CJ):
                at_ps = ppool.tile([P, IPC], BF16, tag="at_ps")
                nc.tensor.transpose(
                    at_ps[:],
                    alpha_h[:, c * P : (c + 1) * P],
                    identb[0:IPC, 0:IPC],
                )
                nc.vector.tensor_copy(
                    alphaT[:, c * IPC : (c + 1) * IPC], at_ps[:]
                )

            # ---- out[i, c*128+p, l] = v[c*128+p, l] * alpha[i, c*128+p]
            # First block is a single row so the store stream starts as
            # early as possible; the rest are IPB-row blocks.
            blocks = [(0, 1), (1, 1), (2, 1), (3, 1)] + [
                (ib, min(IPB, IPC - ib)) for ib in range(4, IPC, IPB)
            ]
            for ib, nb in blocks:
                ot = opool.tile([P, IPB * CJ * L], OUT_DT, tag="ot")
                for t in range(nb):
                    i = ib + t
                    for c in range(CJ):
                        dst = ot[:, (t * CJ + c) * L : (t * CJ + c + 1) * L]
                        src = v_sb[:, c * L : (c + 1) * L]
                        sc = alphaT[:, c * IPC + i : c * IPC + i + 1]
                        if i == 0 and c == 1:
                            # first row: split DVE/ACT so the first store
                            # DMA fires as early as possible (gpsimd is
                            # ~14x slower here and stalls DVE via the
                            # shared SBUF ports - do not use it). Do NOT
                            # split this row into per-chunk DMAs: the
                            # 768B-descriptor DMAs stall the queue head
                            # for ~4us (descriptor-dominated, FIFO ahead
                            # of the big blocks) - measured +6us median.
                            nc.scalar.mul(dst, src, sc)
                        elif (i * CJ + c) % ACT_MOD < ACT_CUT:
                            nc.vector.tensor_scalar_mul(dst, src, sc)
                        else:
                            nc.scalar.mul(dst, src, sc)
                dram_ap = out_d[ib : ib + nb].rearrange(
                    "t (p c) l -> p t c l", p=P, c=CJ
                )
                sb_ap = ot[:, 0 : nb * CJ * L].rearrange(
                    "p (t c l) -> p t c l", t=nb, c=CJ
                )
                # (tried issuing block 0 from the ACT ring to skip the
                # cross-engine sem hop: ACT's issue actually lands ~1us
                # LATER and the ACT ring's first-byte DGE latency is
                # ~1.3us vs SP's ~0.6us -- no gain, reverted)
                nc.sync.dma_start(out=dram_ap, in_=sb_ap)

    nc.compile()
    return nc


def _prep_inputs(h, v, wh, wv, wg):
    """Host-side relayout into the per-core SBUF-friendly layouts."""
    h = np.ascontiguousarray(h, dtype=np.float32)
    v = np.ascontiguousarray(v, dtype=np.float32)
    wh = np.ascontiguousarray(wh, dtype=np.float32)
    wv = np.ascontiguousarray(wv, dtype=np.float32)
    wg = np.ascontiguousarray(wg, dtype=np.float32)

    def bf16(x):
        import ml_dtypes

        return np.ascontiguousarray(x.astype(ml_dtypes.bfloat16))

    # v3 (broadcast source): layout B, v3[p, c*384+l] = v[3p+c, l]
    # so each partition's 3 rows are CONSECUTIVE in the output -> 2.3 KB
    # contiguous HBM runs per store-DMA descriptor row.
    v3 = v.reshape(P, CJ * L)
    # vA (matmul rhs): layout A, vA[p, k*384+l] = v[k*128+p, l]
    vA = v.reshape(CJ, P, L).transpose(1, 0, 2).reshape(P, CJ, L)
    wvT3 = wv.T.reshape(KV, P, E).transpose(1, 0, 2)  # (P, KV, E)
    # fused [wvT chunk | vb chunk] per k
    wvvb = bf16(np.concatenate([wvT3, vA], axis=2).reshape(P, KV * EL))
    wgT3 = wg.T.reshape(KE, P, E).transpose(1, 0, 2).reshape(P, KE * E)
    hwg = bf16(np.concatenate([h.reshape(KE, P).T, wgT3], axis=1))

    in_maps = []
    for m in range(NCORES):
        whm = wh[m * IPC : (m + 1) * IPC]  # (48, 512)
        whT3 = whm.T.reshape(KE, P, IPC).transpose(1, 0, 2).reshape(P, KE * IPC)
        wv3 = bf16(np.concatenate([whT3, v3], axis=1))
        in_maps.append(
            {
                "wvvb": wvvb,
                "hwg": hwg,
                "wv3": wv3,
            }
        )
    return in_maps


_NC_CACHE = []


def _run(inputs: dict, trace: bool = False, **kw):
    if not _NC_CACHE:
        _NC_CACHE.append(_build_nc())
    nc = _NC_CACHE[0]
    in_maps = _prep_inputs(**inputs)
    res = run_bass_kernel_spmd(
        nc, in_maps, core_ids=list(range(NCORES)), trace=trace, **kw
    )
    # 16-bit device output -> f32 on the host (lossless upcast; the
    # device computed every output value, this is just representation).
    out = np.concatenate(
        [np.asarray(r["out"]).astype(np.float32) for r in res.results], axis=0
    )
    return out, res


def kernel(h, v, wh, wv, wg):
    out, _ = _run({"h": h, "v": v, "wh": wh, "wv": wv, "wg": wg})
    return out

